# revision 1
# baseline (speedup 1.0000x reference)
"""Butterfly multiply (n=4096, 12 stages, increasing stride) on 8 Trainium2
NeuronCores.

Math: the 12 butterfly stages factor into
  out = scatter( B-blockdiag @ P-permute( A-blockdiag @ x^T ) )
where stages 0..6 (strides 1..64) compose into 32 dense 128x128 matrices A_o
acting within 128-aligned blocks, and stages 7..11 (strides 128..2048) compose
into 128 dense 32x32 matrices C_i acting across blocks at fixed within-block
index.  Both are composed on the host from the (tiny) twiddle input; the heavy
data (x: 128 MiB) runs through two TensorEngine matmul passes per core.

Layout: the host ships each core's batch shard TRANSPOSED (n on the leading
axis), so the device consumes matmul-ready [n-part, batch] tiles with plain
contiguous DMA loads.  Pass B uses the data as the stationary operand, so its
output comes out batch-major and stores contiguously.

Sharding: batch 8192 split across 8 cores (data parallel), twiddle-derived
matrices replicated.
"""

import os
import sys
import numpy as np

LOG_N = 12
N = 4096
BATCH = 8192
N_CORES = 8
B_CORE = BATCH // N_CORES  # 1024 rows per core

# compute dtype: "fp32" (safe, PE quarter-rate) or "fp16" (fast, ~5e-4 rel err)
COMPUTE = os.environ.get("BUTTERFLY_COMPUTE", "fp16")
BC = 128                                 # pass-B / store granularity
BCO = int(os.environ.get(
    "BUTTERFLY_BCO", "512" if COMPUTE == "fp16" else "256"
))  # pass-A free dim / permute granularity


def _compose_matrices(twiddle):
    """Compose stages 0..6 -> A (32,128,128) and stages 7..11 -> C (128,32,32),
    in float64."""
    tw = np.asarray(twiddle)[0, 0].astype(np.float64)  # (12, 2048, 2, 2)

    A = np.zeros((32, 128, 128))
    A[:, np.arange(128), np.arange(128)] = 1.0
    for idx in range(7):
        s = 1 << idx
        Ar = A.reshape(32, 128 // (2 * s), 2, s, 128)  # (o, dl, k, j, i_in)
        o = np.arange(32)[:, None, None]
        dl = np.arange(128 // (2 * s))[None, :, None]
        j = np.arange(s)[None, None, :]
        m = (o * (64 // s) + dl) * s + j
        t = tw[idx, m]  # (32, dl, j, 2, 2)
        x0, x1 = Ar[:, :, 0], Ar[:, :, 1]
        new0 = t[..., 0, 0:1] * x0 + t[..., 0, 1:2] * x1
        new1 = t[..., 1, 0:1] * x0 + t[..., 1, 1:2] * x1
        A = np.stack([new0, new1], axis=2).reshape(32, 128, 128)

    C = np.zeros((128, 32, 32))
    C[:, np.arange(32), np.arange(32)] = 1.0
    for idx in range(7, 12):
        s = 1 << idx
        sp = s // 128
        Cr = C.reshape(128, 32 // (2 * sp), 2, sp, 32)  # (i, dl, k, ol, o_in)
        i = np.arange(128)[None, None, :]
        dl = np.arange(32 // (2 * sp))[:, None, None]
        ol = np.arange(sp)[None, :, None]
        m = dl * (128 * sp) + 128 * ol + i  # (dl, ol, i)
        t = np.moveaxis(tw[idx, m], 2, 0)  # (i, dl, ol, 2, 2)
        x0, x1 = Cr[:, :, 0], Cr[:, :, 1]
        new0 = t[..., 0, 0:1] * x0 + t[..., 0, 1:2] * x1
        new1 = t[..., 1, 0:1] * x0 + t[..., 1, 1:2] * x1
        C = np.stack([new0, new1], axis=2).reshape(128, 32, 32)

    # AT[o] = A[o].T  (lhsT layout for pass A)
    AT = np.ascontiguousarray(np.transpose(A, (0, 2, 1)))
    # R[tau][u=(a*32+o_in), v=(o_out*4+a)] = C[4*tau+a][o_out, o_in]
    R = np.zeros((32, 128, 128))
    for tau in range(32):
        for a in range(4):
            R[tau, a * 32:(a + 1) * 32, a::4] = C[4 * tau + a].T
    return AT, R


def _build_program(np_dt, mybir_dt, b_core=B_CORE):
    """Trace + compile the per-core Bass program. Returns nc."""
    import concourse.bacc as bacc
    import concourse.tile as tile
    import concourse.mybir as mybir
    from contextlib import ExitStack

    f32 = mybir.dt.float32
    dt = mybir_dt

    nc = bacc.Bacc(
        "TRN2",
        target_bir_lowering=False,
        debug=False,
        enable_asserts=False,
        num_devices=1,
    )
    # x shipped pre-transposed and chunk-contiguous: [n_outer, N, BCO]
    x_ap = nc.dram_tensor(
        "xt", (b_core // BCO, N, BCO), dt, kind="ExternalInput"
    ).ap()
    # AT/R shipped pre-arranged as [k, o*128+m] so the load is a plain 2D copy
    at_ap = nc.dram_tensor("AT", (128, 32 * 128), dt, kind="ExternalInput").ap()
    r_ap = nc.dram_tensor("R", (128, 32 * 128), dt, kind="ExternalInput").ap()
    y_ap = nc.dram_tensor("y", (b_core, N), dt, kind="ExternalOutput").ap()

    n_outer = b_core // BCO
    n_inner = BCO // BC

    with tile.TileContext(nc) as tc, ExitStack() as ctx:
        wpool = ctx.enter_context(tc.tile_pool(name="weights", bufs=1))
        # xT doubles as y1: pass A's copy for block o overwrites the xT slice
        # of block o right after the matmul that consumed it (bufs=2 so the
        # next outer chunk's loads overlap this chunk's tail).
        xT_pool = ctx.enter_context(
            tc.tile_pool(name="xT", bufs=2 if n_outer > 1 else 1)
        )
        z_pool = ctx.enter_context(
            tc.tile_pool(
                name="z",
                bufs=2 if (COMPUTE == "fp16" and n_outer > 1) else 1,
            )
        )
        out_pool = ctx.enter_context(tc.tile_pool(name="outb", bufs=4))
        psA_pool = ctx.enter_context(
            tc.tile_pool(name="psA", bufs=3 if BCO <= 512 else 2, space="PSUM")
        )
        psB_pool = ctx.enter_context(tc.tile_pool(name="psB", bufs=3, space="PSUM"))

        ATw = wpool.tile([128, 32 * 128], dt, tag="ATw")
        Rw = wpool.tile([128, 32 * 128], dt, tag="Rw")
        nc.sync.dma_start(ATw[:], at_ap)
        nc.scalar.dma_start(Rw[:], r_ap)

        for cc in range(n_outer):
            # 1. load transposed chunk: xT free layout o*BCO + b
            #    each per-o source region is fully contiguous (128*BCO elems)
            xT = xT_pool.tile([128, 32 * BCO], dt, tag="xT")
            for o in range(32):
                eng = nc.sync if (o % 2 == 0) else nc.scalar
                eng.dma_start(
                    xT[:, o * BCO:(o + 1) * BCO],
                    x_ap[cc, o * 128:(o + 1) * 128, :],
                )

            # 2. pass A: y1[(o,i'), b] = sum_i A_o[i', i] xT[(o,i), b]
            y1 = xT
            n_sub = max(1, BCO // 512)
            for o in range(32):
                psA = psA_pool.tile([128, BCO], f32, tag="psA")
                for ss in range(n_sub):
                    w = BCO // n_sub
                    nc.tensor.matmul(
                        psA[:, ss * w:(ss + 1) * w],
                        ATw[:, o * 128:(o + 1) * 128],
                        xT[:, o * BCO + ss * w:o * BCO + (ss + 1) * w],
                        start=True,
                        stop=True,
                    )
                nc.any.tensor_copy(y1[:, o * BCO:(o + 1) * BCO], psA[:])

            # 3. permute: z[(a*32+o), tau*BCO+b] = y1[4*tau+a, o*BCO+b]
            #    SWDGE so descriptor-gen rides the otherwise-idle GpSimd Q7s
            z = z_pool.tile([128, 32 * BCO], dt, tag="z")
            perm_mode = os.environ.get("BUTTERFLY_PERM", "sw")
            for tau in range(32):
                if perm_mode == "hw":
                    eng = nc.scalar if (tau % 2 == 0) else nc.sync
                else:
                    eng = nc.gpsimd
                eng.dma_start(
                    z[:, tau * BCO:(tau + 1) * BCO],
                    y1[4 * tau:4 * (tau + 1), :].rearrange("a (o b) -> a o b", b=BCO),
                )

            # 4. pass B (data as stationary operand) + 5. scatter-copy + store
            for jc in range(n_inner):
                outb = out_pool.tile([BC, N], dt, tag="outb")
                outb_re = outb[:].rearrange(
                    "p (op gg tt a) -> gg p tt op a", op=32, gg=8, tt=4, a=4
                )
                for g in range(8):
                    psB = psB_pool.tile([BC, 512], f32, tag="psB")
                    for tt in range(4):
                        tau = 4 * g + tt
                        nc.tensor.matmul(
                            psB[:, tt * 128:(tt + 1) * 128],
                            z[:, tau * BCO + jc * BC:tau * BCO + (jc + 1) * BC],
                            Rw[:, tau * 128:(tau + 1) * 128],
                            start=True,
                            stop=True,
                        )
                    nc.any.tensor_copy(
                        outb_re[g],
                        psB[:].rearrange("p (tt op a) -> p tt op a", tt=4, op=32, a=4),
                    )

                eng = nc.sync if (jc % 2 == 0) else nc.scalar
                eng.dma_start(
                    y_ap[cc * BCO + jc * BC:cc * BCO + (jc + 1) * BC, :], outb[:]
                )

    nc.compile()
    return nc


_CACHE = {}


def _get_program():
    import concourse.mybir as mybir

    key = COMPUTE
    if key not in _CACHE:
        if COMPUTE == "fp16":
            _CACHE[key] = (_build_program(np.float16, mybir.dt.float16), np.float16)
        else:
            _CACHE[key] = (_build_program(np.float32, mybir.dt.float32), np.float32)
    return _CACHE[key]


def run(x, twiddle, trace=False, trace_kwargs=None):
    """Run the butterfly kernel on 8 cores. Returns (out, BassKernelResults)."""
    from concourse.bass_utils import run_bass_kernel_spmd

    nc, np_dt = _get_program()

    AT, R = _compose_matrices(twiddle)
    # [o, k, m] -> [k, o*128+m] (matches the SBUF weight layout)
    ATd = np.ascontiguousarray(AT.transpose(1, 0, 2).reshape(128, 32 * 128)).astype(np_dt)
    Rd = np.ascontiguousarray(R.transpose(1, 0, 2).reshape(128, 32 * 128)).astype(np_dt)

    x = np.asarray(x)
    in_dtype = x.dtype
    xd = x.astype(np_dt)

    in_maps = []
    for c in range(N_CORES):
        shard = xd[c * B_CORE:(c + 1) * B_CORE]
        # [n_outer, N, BCO]: per-chunk transposed, chunk-contiguous
        xtc = np.ascontiguousarray(
            shard.reshape(B_CORE // BCO, BCO, N).transpose(0, 2, 1)
        )
        in_maps.append({"xt": xtc, "AT": ATd, "R": Rd})

    res = run_bass_kernel_spmd(
        nc,
        in_maps,
        core_ids=list(range(N_CORES)),
        trace=trace,
        **(trace_kwargs or {}),
    )
    out = np.concatenate([r["y"] for r in res.results], axis=0)
    return out.astype(in_dtype), res


def kernel(x, twiddle):
    out, _ = run(x, twiddle)
    return out



# revision 4
# speedup vs baseline: 1.5104x; 1.5104x over previous
"""Butterfly multiply (n=4096, 12 stages, increasing stride) on 8 Trainium2
NeuronCores.

Math: the 12 butterfly stages factor into two dense passes
  stage1[i', o, b] = sum_i A_o[i', i] x[o, i, b]      (stages 0..6, within
                                                       128-aligned blocks)
  out[o', i, b]    = sum_o C_i[o', o] stage1[i, o, b]  (stages 7..11, across
                                                       blocks at fixed i)
with A (32 matrices 128x128) and C (128 matrices 32x32) composed on the host
from the tiny twiddle input.

Between the passes the contraction dim must move from the free axis to the
partition axis.  With pass-A output rows ordered m = a*32 + tau (i = 4*tau+a)
and the free axis ordered b-major/o-minor, that swap is exactly a 32x32
block transpose at fixed partition-block a and batch b — which the DVE
StreamTranspose instruction does natively on fully contiguous fp16 APs
(one instruction per half-chunk, ~2x mode).  This replaces the baseline's
SBUF->SBUF DMA permute (9216 x 1KB descriptors, ~70us of DMA engine time).

The input is shipped host-transposed as [chunk, i, o, b] so each per-
partition row is 32KB contiguous in HBM -> 4KB+ DMA packets instead of 1KB.
Stores are [128, 4096] row-contiguous (8KB packets), as in the baseline.

Sharding: batch 8192 split across 8 cores (data parallel), twiddle-derived
matrices replicated.
"""

import numpy as np

LOG_N = 12
N = 4096
BATCH = 8192
N_CORES = 8
B_CORE = BATCH // N_CORES  # 1024 rows per core
BCO = 512                  # chunk batch size (2 chunks per core)
BC = 128                   # pass-B / store batch window


def _compose_matrices(twiddle):
    """Compose stages 0..6 -> A (32,128,128) and stages 7..11 -> C (128,32,32),
    in float64."""
    tw = np.asarray(twiddle)[0, 0].astype(np.float64)  # (12, 2048, 2, 2)

    A = np.zeros((32, 128, 128))
    A[:, np.arange(128), np.arange(128)] = 1.0
    for idx in range(7):
        s = 1 << idx
        Ar = A.reshape(32, 128 // (2 * s), 2, s, 128)  # (o, dl, k, j, i_in)
        o = np.arange(32)[:, None, None]
        dl = np.arange(128 // (2 * s))[None, :, None]
        j = np.arange(s)[None, None, :]
        m = (o * (64 // s) + dl) * s + j
        t = tw[idx, m]  # (32, dl, j, 2, 2)
        x0, x1 = Ar[:, :, 0], Ar[:, :, 1]
        new0 = t[..., 0, 0:1] * x0 + t[..., 0, 1:2] * x1
        new1 = t[..., 1, 0:1] * x0 + t[..., 1, 1:2] * x1
        A = np.stack([new0, new1], axis=2).reshape(32, 128, 128)

    C = np.zeros((128, 32, 32))
    C[:, np.arange(32), np.arange(32)] = 1.0
    for idx in range(7, 12):
        s = 1 << idx
        sp = s // 128
        Cr = C.reshape(128, 32 // (2 * sp), 2, sp, 32)  # (i, dl, k, ol, o_in)
        i = np.arange(128)[None, None, :]
        dl = np.arange(32 // (2 * sp))[:, None, None]
        ol = np.arange(sp)[None, :, None]
        m = dl * (128 * sp) + 128 * ol + i  # (dl, ol, i)
        t = np.moveaxis(tw[idx, m], 2, 0)  # (i, dl, ol, 2, 2)
        x0, x1 = Cr[:, :, 0], Cr[:, :, 1]
        new0 = t[..., 0, 0:1] * x0 + t[..., 0, 1:2] * x1
        new1 = t[..., 1, 0:1] * x0 + t[..., 1, 1:2] * x1
        C = np.stack([new0, new1], axis=2).reshape(128, 32, 32)

    # Pass-A lhsT with output rows permuted: column m = a*32 + tau holds the
    # A_o row i' = 4*tau + a, so SBUF partition m of stage1 is (a, tau).
    m = np.arange(128)
    perm = 4 * (m % 32) + (m // 32)          # i'(m)
    AT = np.transpose(A, (0, 2, 1))[:, :, perm]  # (o, k, m)

    # R[tau][u=(a*32+o_in), v=(o_out*4+a)] = C[4*tau+a][o_out, o_in]
    R = np.zeros((32, 128, 128))
    for tau in range(32):
        for a in range(4):
            R[tau, a * 32:(a + 1) * 32, a::4] = C[4 * tau + a].T
    return AT, R


def _build_program(np_dt, mybir_dt):
    """Trace + compile the per-core Bass program. Returns nc."""
    import concourse.bacc as bacc
    import concourse.tile as tile
    import concourse.mybir as mybir
    from contextlib import ExitStack

    f32 = mybir.dt.float32
    dt = mybir_dt

    nc = bacc.Bacc(
        "TRN2",
        target_bir_lowering=False,
        debug=False,
        enable_asserts=False,
        num_devices=1,
    )
    n_outer = B_CORE // BCO   # 2
    n_inner = BCO // BC       # 4
    CH = 32 * BCO             # 16384 free elems per chunk

    # x shipped pre-transposed: [chunk, i, o*BCO + b] (32KB contiguous rows)
    x_ap = nc.dram_tensor("xt", (n_outer, 128, CH), dt, kind="ExternalInput").ap()
    at_ap = nc.dram_tensor("AT", (128, 32 * 128), dt, kind="ExternalInput").ap()
    r_ap = nc.dram_tensor("R", (128, 32 * 128), dt, kind="ExternalInput").ap()
    y_ap = nc.dram_tensor("y", (B_CORE, N), dt, kind="ExternalOutput").ap()

    with tile.TileContext(nc) as tc, ExitStack() as ctx:
        wpool = ctx.enter_context(tc.tile_pool(name="weights", bufs=1))
        xT_pool = ctx.enter_context(tc.tile_pool(name="xT", bufs=2))
        y1_pool = ctx.enter_context(tc.tile_pool(name="y1", bufs=1))
        z_pool = ctx.enter_context(tc.tile_pool(name="z", bufs=2))
        out_pool = ctx.enter_context(tc.tile_pool(name="outb", bufs=3))
        psA_pool = ctx.enter_context(tc.tile_pool(name="psA", bufs=2, space="PSUM"))
        psB_pool = ctx.enter_context(tc.tile_pool(name="psB", bufs=3, space="PSUM"))

        ATw = wpool.tile([128, 32 * 128], dt, tag="ATw")
        Rw = wpool.tile([128, 32 * 128], dt, tag="Rw")
        nc.sync.dma_start(ATw[:], at_ap)
        nc.scalar.dma_start(Rw[:], r_ap)

        for cc in range(n_outer):
            # 1. load chunk: xT free layout o*BCO + b; 8 DMAs of 4KB rows
            xT = xT_pool.tile([128, CH], dt, tag="xT")
            for s in range(8):
                eng = nc.sync if (s % 2 == 0) else nc.scalar
                w = CH // 8
                eng.dma_start(
                    xT[:, s * w:(s + 1) * w], x_ap[cc, :, s * w:(s + 1) * w]
                )

            # 2. pass A: psA[m=(a,tau), b] per block o -> y1[m, b*32 + o]
            y1 = y1_pool.tile([128, CH], dt, tag="y1")
            y1_re = y1[:].rearrange("p (b oo j) -> oo p b j", oo=16, j=2)
            for op_ in range(16):  # o-pairs
                psA = psA_pool.tile([128, 2 * BCO], f32, tag="psA")
                for j in range(2):
                    o = 2 * op_ + j
                    nc.tensor.matmul(
                        psA[:, j * BCO:(j + 1) * BCO],
                        ATw[:, o * 128:(o + 1) * 128],
                        xT[:, o * BCO:(o + 1) * BCO],
                        start=True,
                        stop=True,
                    )
                nc.any.tensor_copy(
                    y1_re[op_], psA[:].rearrange("p (j b) -> p b j", j=2)
                )

            # 3. permute: 32x32 block transpose (tau <-> o at fixed a, b)
            #    z[a*32+o, b*32+tau] = y1[a*32+tau, b*32+o]
            z = z_pool.tile([128, CH], dt, tag="z")
            nc.vector.transpose(z[:, :CH // 2], y1[:, :CH // 2])
            nc.vector.transpose(z[:, CH // 2:], y1[:, CH // 2:])
            zv = z[:].rearrange("p (b t) -> t p b", t=32)  # [32, 128, BCO]

            # 4. pass B + 5. scatter-copy + store
            for jc in range(n_inner):
                outb = out_pool.tile([BC, N], dt, tag="outb")
                outb_re = outb[:].rearrange(
                    "p (op gg tt a) -> gg p tt op a", op=32, gg=8, tt=4, a=4
                )
                for g in range(8):
                    psB = psB_pool.tile([BC, 512], f32, tag="psB")
                    for tt in range(4):
                        tau = 4 * g + tt
                        nc.tensor.matmul(
                            psB[:, tt * 128:(tt + 1) * 128],
                            zv[tau][:, jc * BC:(jc + 1) * BC],
                            Rw[:, tau * 128:(tau + 1) * 128],
                            start=True,
                            stop=True,
                        )
                    nc.any.tensor_copy(
                        outb_re[g],
                        psB[:].rearrange("p (tt op a) -> p tt op a", tt=4, op=32, a=4),
                    )

                eng = nc.sync if (jc % 2 == 0) else nc.scalar
                eng.dma_start(
                    y_ap[cc * BCO + jc * BC:cc * BCO + (jc + 1) * BC, :], outb[:]
                )

    nc.compile()
    return nc


_CACHE = {}


def _get_program():
    import concourse.mybir as mybir

    if "prog" not in _CACHE:
        _CACHE["prog"] = (_build_program(np.float16, mybir.dt.float16), np.float16)
    return _CACHE["prog"]


def run(x, twiddle, trace=False, trace_kwargs=None):
    """Run the butterfly kernel on 8 cores. Returns (out, BassKernelResults)."""
    from concourse.bass_utils import run_bass_kernel_spmd

    nc, np_dt = _get_program()

    AT, R = _compose_matrices(twiddle)
    # [o, k, m] -> [k, o*128+m] (matches the SBUF weight layout)
    ATd = np.ascontiguousarray(AT.transpose(1, 0, 2).reshape(128, 32 * 128)).astype(np_dt)
    Rd = np.ascontiguousarray(R.transpose(1, 0, 2).reshape(128, 32 * 128)).astype(np_dt)

    x = np.asarray(x)
    in_dtype = x.dtype
    xd = x.astype(np_dt)

    in_maps = []
    for c in range(N_CORES):
        shard = xd[c * B_CORE:(c + 1) * B_CORE]
        # [chunk, i, o, b]: per-partition rows fully contiguous in HBM
        xtc = np.ascontiguousarray(
            shard.reshape(B_CORE // BCO, BCO, 32, 128).transpose(0, 3, 2, 1)
        ).reshape(B_CORE // BCO, 128, 32 * BCO)
        in_maps.append({"xt": xtc, "AT": ATd, "R": Rd})

    res = run_bass_kernel_spmd(
        nc,
        in_maps,
        core_ids=list(range(N_CORES)),
        trace=trace,
        **(trace_kwargs or {}),
    )
    out = np.concatenate([r["y"] for r in res.results], axis=0)
    return out.astype(in_dtype), res


def kernel(x, twiddle):
    out, _ = run(x, twiddle)
    return out


# revision 9
# speedup vs baseline: 1.5203x; 1.0065x over previous
"""Butterfly multiply (n=4096, 12 stages, increasing stride) on 8 Trainium2
NeuronCores.

Math: the 12 butterfly stages factor into two dense passes
  stage1[i', o, b] = sum_i A_o[i', i] x[o, i, b]      (stages 0..6, within
                                                       128-aligned blocks)
  out[o', i, b]    = sum_o C_i[o', o] stage1[i, o, b]  (stages 7..11, across
                                                       blocks at fixed i)
with A (32 matrices 128x128) and C (128 matrices 32x32) composed on the host
from the tiny twiddle input.

Between the passes the contraction dim must move from the free axis to the
partition axis.  With pass-A output rows ordered m = a*32 + tau (i = 4*tau+a)
and the free axis ordered (b-pair, o, b-parity), that swap is exactly a 32x32
block transpose at fixed partition-block a and batch-pair bp — done by the
DVE StreamTranspose instruction on uint32-bitcast views (two fp16 batch
lanes ride in each 4-byte element, halving DVE cycles).

The input is shipped host-transposed as [chunk, i, o, b] so each per-
partition row is 32KB contiguous in HBM -> 8KB DMA packets.  Stores are
[128, 4096] row-contiguous (8KB packets).

Sharding: batch 8192 split across 8 cores (data parallel), twiddle-derived
matrices replicated.
"""

import numpy as np

LOG_N = 12
N = 4096
BATCH = 8192
N_CORES = 8
B_CORE = BATCH // N_CORES  # 1024 rows per core
BCO = 512                  # chunk batch size (2 chunks per core)
BC = 128                   # pass-B / store batch window


def _compose_matrices(twiddle):
    """Compose stages 0..6 -> A (32,128,128) and stages 7..11 -> C (128,32,32),
    in float64."""
    tw = np.asarray(twiddle)[0, 0].astype(np.float64)  # (12, 2048, 2, 2)

    A = np.zeros((32, 128, 128))
    A[:, np.arange(128), np.arange(128)] = 1.0
    for idx in range(7):
        s = 1 << idx
        Ar = A.reshape(32, 128 // (2 * s), 2, s, 128)  # (o, dl, k, j, i_in)
        o = np.arange(32)[:, None, None]
        dl = np.arange(128 // (2 * s))[None, :, None]
        j = np.arange(s)[None, None, :]
        m = (o * (64 // s) + dl) * s + j
        t = tw[idx, m]  # (32, dl, j, 2, 2)
        x0, x1 = Ar[:, :, 0], Ar[:, :, 1]
        new0 = t[..., 0, 0:1] * x0 + t[..., 0, 1:2] * x1
        new1 = t[..., 1, 0:1] * x0 + t[..., 1, 1:2] * x1
        A = np.stack([new0, new1], axis=2).reshape(32, 128, 128)

    C = np.zeros((128, 32, 32))
    C[:, np.arange(32), np.arange(32)] = 1.0
    for idx in range(7, 12):
        s = 1 << idx
        sp = s // 128
        Cr = C.reshape(128, 32 // (2 * sp), 2, sp, 32)  # (i, dl, k, ol, o_in)
        i = np.arange(128)[None, None, :]
        dl = np.arange(32 // (2 * sp))[:, None, None]
        ol = np.arange(sp)[None, :, None]
        m = dl * (128 * sp) + 128 * ol + i  # (dl, ol, i)
        t = np.moveaxis(tw[idx, m], 2, 0)  # (i, dl, ol, 2, 2)
        x0, x1 = Cr[:, :, 0], Cr[:, :, 1]
        new0 = t[..., 0, 0:1] * x0 + t[..., 0, 1:2] * x1
        new1 = t[..., 1, 0:1] * x0 + t[..., 1, 1:2] * x1
        C = np.stack([new0, new1], axis=2).reshape(128, 32, 32)

    # Pass-A lhsT with output rows permuted: column m = a*32 + tau holds the
    # A_o row i' = 4*tau + a, so SBUF partition m of stage1 is (a, tau).
    m = np.arange(128)
    perm = 4 * (m % 32) + (m // 32)          # i'(m)
    AT = np.transpose(A, (0, 2, 1))[:, :, perm]  # (o, k, m)

    # R[tau][u=(a*32+o_in), v=(o_out*4+a)] = C[4*tau+a][o_out, o_in]
    R = np.zeros((32, 128, 128))
    for tau in range(32):
        for a in range(4):
            R[tau, a * 32:(a + 1) * 32, a::4] = C[4 * tau + a].T
    return AT, R


def _build_program(np_dt, mybir_dt):
    """Trace + compile the per-core Bass program. Returns nc."""
    import concourse.bacc as bacc
    import concourse.tile as tile
    import concourse.mybir as mybir
    from contextlib import ExitStack

    f32 = mybir.dt.float32
    u32 = mybir.dt.uint32
    dt = mybir_dt

    nc = bacc.Bacc(
        "TRN2",
        target_bir_lowering=False,
        debug=False,
        enable_asserts=False,
        num_devices=1,
    )
    n_outer = B_CORE // BCO   # 2
    n_inner = BCO // BC       # 4
    CH = 32 * BCO             # 16384 free elems per chunk

    # x shipped pre-transposed: [chunk, i, o*BCO + b] (32KB contiguous rows)
    x_ap = nc.dram_tensor("xt", (n_outer, 128, CH), dt, kind="ExternalInput").ap()
    at_ap = nc.dram_tensor("AT", (128, 32 * 128), dt, kind="ExternalInput").ap()
    r_ap = nc.dram_tensor("R", (128, 32 * 128), dt, kind="ExternalInput").ap()
    y_ap = nc.dram_tensor("y", (B_CORE, N), dt, kind="ExternalOutput").ap()

    with tile.TileContext(nc) as tc, ExitStack() as ctx:
        wpool = ctx.enter_context(tc.tile_pool(name="weights", bufs=1))
        # xT and y1 share one 3-buffer pool (xT(c), y1(c), xT(c+1) coexist)
        xy_pool = ctx.enter_context(tc.tile_pool(name="xy", bufs=3))
        z_pool = ctx.enter_context(tc.tile_pool(name="z", bufs=2))
        out_pool = ctx.enter_context(tc.tile_pool(name="outb", bufs=3))
        psA_pool = ctx.enter_context(tc.tile_pool(name="psA", bufs=2, space="PSUM"))
        psB_pool = ctx.enter_context(tc.tile_pool(name="psB", bufs=2, space="PSUM"))

        ATw = wpool.tile([128, 32 * 128], dt, tag="ATw")
        Rw = wpool.tile([128, 32 * 128], dt, tag="Rw")
        nc.sync.dma_start(ATw[:], at_ap)
        nc.sync.dma_start(Rw[:], r_ap)

        # copy on a chosen engine (weighted rotation: Act fast, Pool ~0.6x,
        # DVE busy with transposes)
        COPY = mybir.ActivationFunctionType.Copy

        def _copy(name, out, in_):
            if name == "s":
                nc.scalar.activation(out, in_, COPY)
            elif name == "g":
                nc.gpsimd.tensor_copy(out, in_)
            else:
                nc.vector.tensor_copy(out, in_)

        # gpsimd cannot read PSUM, so PSUM->SBUF copies go to Act ("s")
        # and DVE ("v") only; DVE also runs the stream transposes.
        psA_rot = "svssvssvssvssvsv"   # 10 s, 6 v
        psB_rot = "vssvsvssvsvssvss"   # 10 s, 6 v

        zs = [None] * n_outer

        def emit_load(cc):
            xT = xy_pool.tile([128, CH], dt, tag="xy")
            for s in range(4):
                w = CH // 4
                nc.sync.dma_start(
                    xT[:, s * w:(s + 1) * w], x_ap[cc, :, s * w:(s + 1) * w]
                )
            return xT

        def emit_passA(cc, xT):
            # psA[m=(a,tau), b] per block o; y1 free: bp*64 + o*2 + par
            y1 = xy_pool.tile([128, CH], dt, tag="xy")
            y1_re = y1[:].rearrange(
                "p (bp oo j par) -> oo p bp j par", oo=16, j=2, par=2
            )
            for op_ in range(16):  # o-pairs
                psA = psA_pool.tile([128, 2 * BCO], f32, tag="psA")
                for j in range(2):
                    o = 2 * op_ + j
                    nc.tensor.matmul(
                        psA[:, j * BCO:(j + 1) * BCO],
                        ATw[:, o * 128:(o + 1) * 128],
                        xT[:, o * BCO:(o + 1) * BCO],
                        start=True,
                        stop=True,
                    )
                _copy(
                    psA_rot[op_],
                    y1_re[op_],
                    psA[:].rearrange("p (j bp par) -> p bp j par", j=2, par=2),
                )
            return y1

        def emit_transpose(cc, y1):
            # permute: 32x32 block transpose (tau <-> o at fixed a, bp) on
            # uint32 views (fp16 pair per element):
            # z[a*32+o, bp*32+tau] = y1[a*32+tau, bp*32+o]   (uint32 idx)
            z = z_pool.tile([128, CH], dt, tag="z")
            y1u = y1[:].bitcast(u32)
            # out AP enumerates (bp, tau) but scatters tau-major in memory,
            # so z ends up [u, tau*BCO + b] and pass-B lhsT slices are
            # contiguous single-dim APs.
            zu = z[:].bitcast(u32).rearrange("p (t bp) -> p bp t", t=32)
            for q in range(8):
                w = CH // 16 // 32  # bp-groups per eighth (32)
                nc.vector.transpose(
                    zu[:, q * w:(q + 1) * w, :],
                    y1u[:, q * w * 32:(q + 1) * w * 32],
                )
            zs[cc] = z

        def emit_passB(cc):
            # z free layout: tau*BCO + b
            zv = zs[cc]
            for jc in range(n_inner):
                outb = out_pool.tile([BC, N], dt, tag="outb")
                outb_re = outb[:].rearrange(
                    "p (op g4 gg tt a) -> g4 p gg tt op a",
                    op=32, g4=4, gg=2, tt=4, a=4,
                )
                for g4 in range(4):
                    psB = psB_pool.tile([BC, 1024], f32, tag="psB")
                    for gg in range(2):
                        for tt in range(4):
                            tau = 4 * (2 * g4 + gg) + tt
                            nc.tensor.matmul(
                                psB[:, gg * 512 + tt * 128:gg * 512 + (tt + 1) * 128],
                                zv[:, tau * BCO + jc * BC:tau * BCO + (jc + 1) * BC],
                                Rw[:, tau * 128:(tau + 1) * 128],
                                start=True,
                                stop=True,
                            )
                    _copy(
                        psB_rot[4 * jc + g4],
                        outb_re[g4],
                        psB[:].rearrange(
                            "p (gg tt op a) -> p gg tt op a", gg=2, tt=4, op=32, a=4
                        ),
                    )

                nc.sync.dma_start(
                    y_ap[cc * BCO + jc * BC:cc * BCO + (jc + 1) * BC, :], outb[:]
                )

        # pipeline: PE does passA(1) while DVE transposes chunk 0, and DVE
        # transposes chunk 1 while PE runs passB(0) — no PE idle gaps.
        xT0 = emit_load(0)
        y10 = emit_passA(0, xT0)
        emit_transpose(0, y10)
        xT1 = emit_load(1)
        y11 = emit_passA(1, xT1)
        emit_passB(0)
        emit_transpose(1, y11)
        emit_passB(1)

    nc.compile()
    return nc


_CACHE = {}


def _get_program():
    import concourse.mybir as mybir

    if "prog" not in _CACHE:
        _CACHE["prog"] = (_build_program(np.float16, mybir.dt.float16), np.float16)
    return _CACHE["prog"]


def run(x, twiddle, trace=False, trace_kwargs=None):
    """Run the butterfly kernel on 8 cores. Returns (out, BassKernelResults)."""
    from concourse.bass_utils import run_bass_kernel_spmd

    nc, np_dt = _get_program()

    AT, R = _compose_matrices(twiddle)
    # [o, k, m] -> [k, o*128+m] (matches the SBUF weight layout)
    ATd = np.ascontiguousarray(AT.transpose(1, 0, 2).reshape(128, 32 * 128)).astype(np_dt)
    Rd = np.ascontiguousarray(R.transpose(1, 0, 2).reshape(128, 32 * 128)).astype(np_dt)

    x = np.asarray(x)
    in_dtype = x.dtype
    xd = x.astype(np_dt)

    in_maps = []
    for c in range(N_CORES):
        shard = xd[c * B_CORE:(c + 1) * B_CORE]
        # [chunk, i, o, b]: per-partition rows fully contiguous in HBM
        xtc = np.ascontiguousarray(
            shard.reshape(B_CORE // BCO, BCO, 32, 128).transpose(0, 3, 2, 1)
        ).reshape(B_CORE // BCO, 128, 32 * BCO)
        in_maps.append({"xt": xtc, "AT": ATd, "R": Rd})

    res = run_bass_kernel_spmd(
        nc,
        in_maps,
        core_ids=list(range(N_CORES)),
        trace=trace,
        **(trace_kwargs or {}),
    )
    out = np.concatenate([r["y"] for r in res.results], axis=0)
    return out.astype(in_dtype), res


def kernel(x, twiddle):
    out, _ = run(x, twiddle)
    return out


# revision 10
# speedup vs baseline: 1.5591x; 1.0255x over previous
"""Butterfly multiply (n=4096, 12 stages, increasing stride) on 8 Trainium2
NeuronCores.

Math: the 12 butterfly stages factor into two dense passes
  stage1[i', o, b] = sum_i A_o[i', i] x[o, i, b]      (stages 0..6, within
                                                       128-aligned blocks)
  out[o', i, b]    = sum_o C_i[o', o] stage1[i, o, b]  (stages 7..11, across
                                                       blocks at fixed i)
with A (32 matrices 128x128) and C (128 matrices 32x32) composed on the host
from the tiny twiddle input.

Between the passes the contraction dim must move from the free axis to the
partition axis.  With pass-A output rows ordered m = a*32 + tau (i = 4*tau+a)
and the free axis ordered (b-pair, o, b-parity), that swap is exactly a 32x32
block transpose at fixed partition-block a and batch-pair bp — done by the
DVE StreamTranspose instruction on uint32-bitcast views (two fp16 batch
lanes per 4-byte element).  The transpose output AP scatters tau-major so
pass-B stationary slices are contiguous.

Batch is processed in 4 chunks of 256 with a software-pipelined emission
order (load / passA / transpose / passB overlap across chunks).  Loads ride
the sync queue, stores + pass-B weights the scalar queue.  PSUM->SBUF
copies are balanced between the Act and DVE engines (gpsimd cannot read
PSUM).

Sharding: batch 8192 split across 8 cores (data parallel), twiddle-derived
matrices replicated.
"""

import numpy as np

LOG_N = 12
N = 4096
BATCH = 8192
N_CORES = 8
B_CORE = BATCH // N_CORES  # 1024 rows per core
BCO = 256                  # chunk batch size (4 chunks per core)
BC = 128                   # pass-B / store batch window


def _compose_matrices(twiddle):
    """Compose stages 0..6 -> A (32,128,128) and stages 7..11 -> C (128,32,32),
    in float64."""
    tw = np.asarray(twiddle)[0, 0].astype(np.float64)  # (12, 2048, 2, 2)

    A = np.zeros((32, 128, 128))
    A[:, np.arange(128), np.arange(128)] = 1.0
    for idx in range(7):
        s = 1 << idx
        Ar = A.reshape(32, 128 // (2 * s), 2, s, 128)  # (o, dl, k, j, i_in)
        o = np.arange(32)[:, None, None]
        dl = np.arange(128 // (2 * s))[None, :, None]
        j = np.arange(s)[None, None, :]
        m = (o * (64 // s) + dl) * s + j
        t = tw[idx, m]  # (32, dl, j, 2, 2)
        x0, x1 = Ar[:, :, 0], Ar[:, :, 1]
        new0 = t[..., 0, 0:1] * x0 + t[..., 0, 1:2] * x1
        new1 = t[..., 1, 0:1] * x0 + t[..., 1, 1:2] * x1
        A = np.stack([new0, new1], axis=2).reshape(32, 128, 128)

    C = np.zeros((128, 32, 32))
    C[:, np.arange(32), np.arange(32)] = 1.0
    for idx in range(7, 12):
        s = 1 << idx
        sp = s // 128
        Cr = C.reshape(128, 32 // (2 * sp), 2, sp, 32)  # (i, dl, k, ol, o_in)
        i = np.arange(128)[None, None, :]
        dl = np.arange(32 // (2 * sp))[:, None, None]
        ol = np.arange(sp)[None, :, None]
        m = dl * (128 * sp) + 128 * ol + i  # (dl, ol, i)
        t = np.moveaxis(tw[idx, m], 2, 0)  # (i, dl, ol, 2, 2)
        x0, x1 = Cr[:, :, 0], Cr[:, :, 1]
        new0 = t[..., 0, 0:1] * x0 + t[..., 0, 1:2] * x1
        new1 = t[..., 1, 0:1] * x0 + t[..., 1, 1:2] * x1
        C = np.stack([new0, new1], axis=2).reshape(128, 32, 32)

    # Pass-A lhsT with output rows permuted: column m = a*32 + tau holds the
    # A_o row i' = 4*tau + a, so SBUF partition m of stage1 is (a, tau).
    m = np.arange(128)
    perm = 4 * (m % 32) + (m // 32)          # i'(m)
    AT = np.transpose(A, (0, 2, 1))[:, :, perm]  # (o, k, m)

    # R[tau][u=(a*32+o_in), v=(o_out*4+a)] = C[4*tau+a][o_out, o_in]
    R = np.zeros((32, 128, 128))
    for tau in range(32):
        for a in range(4):
            R[tau, a * 32:(a + 1) * 32, a::4] = C[4 * tau + a].T
    return AT, R


def _build_program(np_dt, mybir_dt):
    """Trace + compile the per-core Bass program. Returns nc."""
    import concourse.bacc as bacc
    import concourse.tile as tile
    import concourse.mybir as mybir
    from contextlib import ExitStack

    f32 = mybir.dt.float32
    u32 = mybir.dt.uint32
    dt = mybir_dt

    nc = bacc.Bacc(
        "TRN2",
        target_bir_lowering=False,
        debug=False,
        enable_asserts=False,
        num_devices=1,
    )
    n_outer = B_CORE // BCO   # 4
    n_inner = BCO // BC       # 2
    CH = 32 * BCO             # 8192 free elems per chunk

    # x shipped pre-transposed: [chunk, i, o*BCO + b] (16KB contiguous rows)
    x_ap = nc.dram_tensor("xt", (n_outer, 128, CH), dt, kind="ExternalInput").ap()
    at_ap = nc.dram_tensor("AT", (128, 32 * 128), dt, kind="ExternalInput").ap()
    r_ap = nc.dram_tensor("R", (128, 32 * 128), dt, kind="ExternalInput").ap()
    y_ap = nc.dram_tensor("y", (B_CORE, N), dt, kind="ExternalOutput").ap()

    with tile.TileContext(nc) as tc, ExitStack() as ctx:
        wpool = ctx.enter_context(tc.tile_pool(name="weights", bufs=1))
        # xT and y1 share one ring (lifetimes interleave across chunks)
        xy_pool = ctx.enter_context(tc.tile_pool(name="xy", bufs=6))
        z_pool = ctx.enter_context(tc.tile_pool(name="z", bufs=3))
        out_pool = ctx.enter_context(tc.tile_pool(name="outb", bufs=4))
        psA_pool = ctx.enter_context(tc.tile_pool(name="psA", bufs=2, space="PSUM"))
        psB_pool = ctx.enter_context(tc.tile_pool(name="psB", bufs=2, space="PSUM"))

        ATw = wpool.tile([128, 32 * 128], dt, tag="ATw")
        Rw = wpool.tile([128, 32 * 128], dt, tag="Rw")
        nc.sync.dma_start(ATw[:], at_ap)     # sync queue: needed first
        nc.scalar.dma_start(Rw[:], r_ap)     # scalar queue: needed at passB

        # gpsimd cannot read PSUM: PSUM->SBUF copies go to Act ("s") and
        # DVE ("v"); DVE also runs the stream transposes.
        COPY = mybir.ActivationFunctionType.Copy

        def _copy(name, out, in_):
            if name == "s":
                nc.scalar.activation(out, in_, COPY)
            else:
                nc.vector.tensor_copy(out, in_)

        psA_rot = "svsssvss"   # per chunk: 6 s, 2 v
        psB_rot = "vssvssvs"   # per chunk: 5 s, 3 v

        xTs = [None] * n_outer
        y1s = [None] * n_outer
        zs = [None] * n_outer

        def emit_load(cc):
            xT = xy_pool.tile([128, CH], dt, tag="xy")
            for s in range(2):
                w = CH // 2
                nc.sync.dma_start(
                    xT[:, s * w:(s + 1) * w], x_ap[cc, :, s * w:(s + 1) * w]
                )
            xTs[cc] = xT

        def emit_passA(cc):
            # psA[m=(a,tau), b] per block o; y1 free: bp*64 + o*2 + par
            xT = xTs[cc]
            y1 = xy_pool.tile([128, CH], dt, tag="xy")
            y1_re = y1[:].rearrange(
                "p (bp oq j par) -> oq p bp j par", oq=8, j=4, par=2
            )
            for oq in range(8):  # o-quads
                psA = psA_pool.tile([128, 4 * BCO], f32, tag="psA")
                for j in range(4):
                    o = 4 * oq + j
                    nc.tensor.matmul(
                        psA[:, j * BCO:(j + 1) * BCO],
                        ATw[:, o * 128:(o + 1) * 128],
                        xT[:, o * BCO:(o + 1) * BCO],
                        start=True,
                        stop=True,
                    )
                _copy(
                    psA_rot[oq],
                    y1_re[oq],
                    psA[:].rearrange("p (j bp par) -> p bp j par", j=4, par=2),
                )
            y1s[cc] = y1

        def emit_transpose(cc):
            # permute: 32x32 block transpose (tau <-> o at fixed a, bp) on
            # uint32 views (fp16 pair per element):
            #   in stream (bp, o), out stream (bp, tau) scattered tau-major
            # so z fp16 layout is tau*BCO + b  (b = 2*bp + par).
            z = z_pool.tile([128, CH], dt, tag="z")
            y1u = y1s[cc][:].bitcast(u32)
            zu = z[:].bitcast(u32).rearrange("p (t bp) -> p bp t", t=32)
            nbp = CH // 64  # bp count per chunk (128)
            for q in range(4):
                w = nbp // 4
                nc.vector.transpose(
                    zu[:, q * w:(q + 1) * w, :],
                    y1u[:, q * w * 32:(q + 1) * w * 32],
                )
            zs[cc] = z

        def emit_passB(cc):
            z = zs[cc]
            for jc in range(n_inner):
                outb = out_pool.tile([BC, N], dt, tag="outb")
                outb_re = outb[:].rearrange(
                    "p (op g4 gg tt a) -> g4 p gg tt op a",
                    op=32, g4=4, gg=2, tt=4, a=4,
                )
                for g4 in range(4):
                    psB = psB_pool.tile([BC, 1024], f32, tag="psB")
                    for gg in range(2):
                        for tt in range(4):
                            tau = 4 * (2 * g4 + gg) + tt
                            nc.tensor.matmul(
                                psB[:, gg * 512 + tt * 128:gg * 512 + (tt + 1) * 128],
                                z[:, tau * BCO + jc * BC:tau * BCO + (jc + 1) * BC],
                                Rw[:, tau * 128:(tau + 1) * 128],
                                start=True,
                                stop=True,
                            )
                    _copy(
                        psB_rot[4 * jc + g4],
                        outb_re[g4],
                        psB[:].rearrange(
                            "p (gg tt op a) -> p gg tt op a", gg=2, tt=4, op=32, a=4
                        ),
                    )

                nc.scalar.dma_start(
                    y_ap[cc * BCO + jc * BC:cc * BCO + (jc + 1) * BC, :], outb[:]
                )

        # software pipeline: transposes (DVE) and passB(c-1) overlap
        # passA(c+1) on the PE; loads all queue on sync in chunk order.
        for cc in range(n_outer):
            emit_load(cc)
        emit_passA(0)
        emit_transpose(0)
        emit_passA(1)
        emit_passB(0)
        emit_transpose(1)
        emit_passA(2)
        emit_passB(1)
        emit_transpose(2)
        emit_passA(3)
        emit_passB(2)
        emit_transpose(3)
        emit_passB(3)

    nc.compile()
    return nc


_CACHE = {}


def _get_program():
    import concourse.mybir as mybir

    if "prog" not in _CACHE:
        _CACHE["prog"] = (_build_program(np.float16, mybir.dt.float16), np.float16)
    return _CACHE["prog"]


def run(x, twiddle, trace=False, trace_kwargs=None):
    """Run the butterfly kernel on 8 cores. Returns (out, BassKernelResults)."""
    from concourse.bass_utils import run_bass_kernel_spmd

    nc, np_dt = _get_program()

    AT, R = _compose_matrices(twiddle)
    # [o, k, m] -> [k, o*128+m] (matches the SBUF weight layout)
    ATd = np.ascontiguousarray(AT.transpose(1, 0, 2).reshape(128, 32 * 128)).astype(np_dt)
    Rd = np.ascontiguousarray(R.transpose(1, 0, 2).reshape(128, 32 * 128)).astype(np_dt)

    x = np.asarray(x)
    in_dtype = x.dtype
    xd = x.astype(np_dt)

    in_maps = []
    for c in range(N_CORES):
        shard = xd[c * B_CORE:(c + 1) * B_CORE]
        # [chunk, i, o, b]: per-partition rows fully contiguous in HBM
        xtc = np.ascontiguousarray(
            shard.reshape(B_CORE // BCO, BCO, 32, 128).transpose(0, 3, 2, 1)
        ).reshape(B_CORE // BCO, 128, 32 * BCO)
        in_maps.append({"xt": xtc, "AT": ATd, "R": Rd})

    res = run_bass_kernel_spmd(
        nc,
        in_maps,
        core_ids=list(range(N_CORES)),
        trace=trace,
        **(trace_kwargs or {}),
    )
    out = np.concatenate([r["y"] for r in res.results], axis=0)
    return out.astype(in_dtype), res


def kernel(x, twiddle):
    out, _ = run(x, twiddle)
    return out
